# revision 26
# baseline (speedup 1.0000x reference)
"""Trainium2 Bass kernel for nn_CausalSelfAttention (BitLinear QKV/O + RoPE + causal attn).

Sharding: head-parallel, 2 heads x 2 batches per core; all matmul operands bf16
(fp32 PSUM accum). Single fused software pipeline: each attention chunk's
j-loop (scores -> exp -> E@V) is interleaved with "filler" work -- the next
slab's Q/K/V projections + RoPE and the previous chunk's output projection --
so the PE stream never drains. V is produced pre-transposed ([token, dim]) by
restructuring its projection (x-chunk as lhsT), eliminating PE transposes.
Diagonal (masked) tiles run first in reversed-j order so their exp+mask chain
hides under the pipeline fill. Q->K share one PSUM bank sequentially; scores
use a 3-deep PSUM rotation; exp runs on ACT; PSUM evacuation copies are
balanced ACT/DVE; RoPE cos/sin multiplies run on Pool. Partial outputs are
written bf16 and summed across cores on the host.
"""
import sys

sys.path.insert(0, "/opt/trn_rl_repo")

from collections import deque

import numpy as np

GROUP = 128
N_HEADS = 16
EPS = 1e-8
B, T, C = 2, 2048, 1024
HD = 64
N_CORES = 8
HPC = N_HEADS // N_CORES  # 2 heads per core
PIPE = 3


# ---------------------------------------------------------------- host prep
def _ternary_quantize(w):
    O, I = w.shape
    g = w.reshape(O, I // GROUP, GROUP).astype(np.float32)
    scale = np.maximum(np.mean(np.abs(g), axis=-1, keepdims=True), EPS).astype(
        np.float32
    )
    wn = g / scale
    q = np.where(wn > 0.5, 1.0, np.where(wn < -0.5, -1.0, 0.0)).astype(np.float32)
    return (q * scale).reshape(O, I).astype(np.float32)


def _np_bf16():
    import concourse.mybir as mybir

    return np.dtype(mybir.dt.np(mybir.dt.bfloat16))


def _make_core_inputs(x, wq, wk, wv, wo, rope_cos, rope_sin):
    """Returns list of 8 per-core input dicts (bf16 device layouts)."""
    bf = _np_bf16()
    x = np.ascontiguousarray(x.astype(np.float32).reshape(B * T, C))
    wq_q = _ternary_quantize(wq) * np.float32(HD**-0.5)  # fold attn scale
    wk_q = _ternary_quantize(wk)
    wv_q = _ternary_quantize(wv)
    wo_q = _ternary_quantize(wo)

    xT = x.T  # [1024 c, 4096 t]
    xt_slab = np.ascontiguousarray(
        xT.reshape(8, 128, 8, 512).transpose(2, 1, 0, 3)
    ).astype(bf)  # [s, p, cc, u]

    cosT = rope_cos.astype(np.float32).T  # [32, 2048]
    sinT = rope_sin.astype(np.float32).T
    cos_t = np.tile(cosT, (4, 1)).astype(bf)
    sin_t = np.concatenate([-sinT, sinT, -sinT, sinT], axis=0).astype(bf)
    tri = (np.arange(128)[None, :] >= np.arange(128)[:, None]).astype(bf)
    # partition-swap matrix: out = PT^T @ in, out[m] = in[sigma(m)],
    # sigma swaps 32-row halves within each 64-row head block.
    m = np.arange(128)
    sigma = np.where(m % 64 < 32, m + 32, m - 32)
    PT = np.zeros((128, 128), np.float32)
    PT[sigma, m] = 1.0
    PT = PT.astype(bf)

    maps = []
    for core in range(N_CORES):
        r0 = core * HPC * HD
        rows = slice(r0, r0 + HPC * HD)

        def w_lhsT(w_qq):
            wsT = w_qq[rows, :].T  # [1024 in, 128 d]
            return np.ascontiguousarray(
                wsT.reshape(8, 128, 128).transpose(1, 0, 2)
            ).astype(bf)

        maps.append(
            {
                "xt": xt_slab,
                "wqT": w_lhsT(wq_q),
                "wkT": w_lhsT(wk_q),
                "wvT": w_lhsT(wv_q),
                "woC": np.ascontiguousarray(wo_q[:, rows].T).astype(bf),
                "cos_t": cos_t,
                "sin_t": sin_t,
                "tri": tri,
                "PT": PT,
                "ones1": np.ones((1, 64), np.float32).astype(bf),
            }
        )
    return maps


# ---------------------------------------------------------------- BIR post-pass
def _split_excess_waits(nc, max_waits=1):
    """walrus CoreV3 codegen rejects instructions with >1 sem wait; split the
    excess into preceding NoOps on the same engine."""
    import concourse.mybir as mybir

    for f in nc.m.functions:
        for bb in f.blocks:
            insts = bb.instructions
            i = 0
            while i < len(insts):
                ins = insts[i]
                si = ins.sync_info
                if si is not None and si.on_wait and len(si.on_wait) > max_waits:
                    waits = list(si.on_wait)
                    si.on_wait = waits[:max_waits]
                    rest = waits[max_waits:]
                    new_ops = []
                    for j in range(0, len(rest), max_waits):
                        new_ops.append(
                            mybir.InstNoOp(
                                name=nc.get_next_instruction_name(),
                                sync_info=mybir.SyncInfo(
                                    on_wait=rest[j : j + max_waits], on_update=[]
                                ),
                                bass_nofuse=True,
                                engine=ins.engine,
                            )
                        )
                    insts[i:i] = new_ops
                    i += len(new_ops)
                i += 1


# ---------------------------------------------------------------- device kernel
class _Balancer:
    """Greedy least-loaded assignment of PSUM-evacuation copies to ACT/DVE."""

    def __init__(self):
        self.busy = {"act": 0.0, "dve": 0.0}

    @staticmethod
    def cost(eng, free):
        if eng == "act":
            return free * 0.833 + 185.0
        return free * 1.042 + 125.0

    def charge(self, eng, free):
        self.busy[eng] += self.cost(eng, free)
        return eng

    def pick(self, free, bias=None):
        bias = bias or {}
        best = min(
            ("act", "dve"),
            key=lambda e: self.busy[e] + self.cost(e, free) + bias.get(e, 0.0),
        )
        return self.charge(best, free)


def _emit(nc, tc, d):
    import concourse.mybir as mybir
    from concourse.bass import ds

    f32 = mybir.dt.float32
    bf16 = mybir.dt.bfloat16
    AF = mybir.ActivationFunctionType
    OP = mybir.AluOpType
    bal = _Balancer()

    def emit_copy(dst, src, free, bias=None):
        eng = bal.pick(free, bias)
        if eng == "act":
            nc.scalar.copy(dst, src)
        else:
            nc.vector.tensor_copy(dst, src)

    with nc.allow_low_precision(
        reason="bf16 matmul operands; fp32 accum in PSUM"
    ), tc.tile_pool(name="const", bufs=1) as cp, tc.tile_pool(
        name="persist", bufs=1
    ) as pp, tc.tile_pool(name="xt", bufs=3) as xtp, tc.tile_pool(
        name="raw", bufs=2
    ) as rawp, tc.tile_pool(name="rope", bufs=2) as ropep, tc.tile_pool(
        name="E", bufs=6
    ) as ep, tc.tile_pool(name="rc", bufs=2) as rcp, tc.tile_pool(
        name="rb", bufs=2
    ) as rbp, tc.tile_pool(name="ob", bufs=2) as obp, tc.tile_pool(
        name="prj", bufs=1, space="PSUM"
    ) as prjp, tc.tile_pool(name="vop", bufs=2, space="PSUM") as vopp, tc.tile_pool(
        name="yp", bufs=1, space="PSUM"
    ) as ypp, tc.tile_pool(name="sp", bufs=3, space="PSUM") as spp:
        # ---- constants / inputs: K weights + x first (K projects first),
        # spread across the SP/ACT/DVE DMA rings so issue overhead overlaps
        wk_t = cp.tile([128, 8, 128], bf16)
        nc.sync.dma_start(wk_t[:, 0:2, :], d["wkT"][:, 0:2, :])
        xt0 = xtp.tile([128, 8, 512], bf16, tag="xt", name="xt0")
        nc.sync.dma_start(xt0[:, 0:2, :], d["xt"][0][:, 0:2, :])
        nc.sync.dma_start(wk_t[:, 2:8, :], d["wkT"][:, 2:8, :])
        nc.sync.dma_start(xt0[:, 2:5, :], d["xt"][0][:, 2:5, :])
        wq_t = cp.tile([128, 8, 128], bf16)
        nc.scalar.dma_start(wq_t[:], d["wqT"])
        wv_t = cp.tile([128, 8, 128], bf16)
        nc.scalar.dma_start(wv_t[:], d["wvT"])
        nc.sync.dma_start(xt0[:, 5:8, :], d["xt"][0][:, 5:8, :])
        xt1 = xtp.tile([128, 8, 512], bf16, tag="xt", name="xt1")
        nc.sync.dma_start(xt1[:], d["xt"][1])
        PT_t = cp.tile([128, 128], bf16)
        nc.scalar.dma_start(PT_t[:], d["PT"])
        cos_sb = cp.tile([128, 2048], bf16)
        nc.scalar.dma_start(cos_sb[:], d["cos_t"])
        sin_sb = cp.tile([128, 2048], bf16)
        nc.scalar.dma_start(sin_sb[:], d["sin_t"])
        tri_t = cp.tile([128, 128], bf16)
        nc.scalar.dma_start(tri_t[:], d["tri"])
        woC = cp.tile([128, 1024], bf16)
        nc.scalar.dma_start(woC[:], d["woC"])
        woC_lo = cp.tile([64, 1024], bf16)
        nc.scalar.dma_start(woC_lo[:], d["woC"][64:128, :])
        ones1 = cp.tile([1, 64], bf16)
        nc.scalar.dma_start(ones1[:], d["ones1"])

        qT = pp.tile([128, 4096], bf16)
        kT = pp.tile([128, 4096], bf16)
        v_sb = pp.tile([128, 64, 65], bf16)
        nc.gpsimd.memset(v_sb[:, :, 64:65], 1.0)  # denominator ones column
        y2 = pp.tile([128, 4096], bf16)
        y2B = pp.tile([64, 4096], bf16)

        xt_tiles = {0: xt0, 1: xt1}

        def load_slab(s):
            t = xtp.tile([128, 8, 512], bf16, tag="xt", name="xt%d" % s)
            nc.sync.dma_start(t[:], d["xt"][s])
            xt_tiles[s] = t

        filler = deque()

        def pump(k=1):
            for _ in range(k):
                if filler:
                    filler.popleft()()

        # ------------------------------------------------ projection closures
        def make_proj(b, ci):
            """Closure list: K then Q proj (shared PSUM tag, sequential), V^T
            proj, RoPE. Emitted as filler inside the previous attention
            section. K first so its longer RoPE chain hides under Q/V mms."""
            s = b * 4 + ci
            cls = []
            if s + 2 < 8:
                cls.append(lambda s2=s + 2: load_slab(s2))

            dcol = ds(s * 512, 512)
            lcol = ds(ci * 512, 512)

            def qk_closures(nm, w_t, dest):
                ps = prjp.tile([128, 512], f32, tag="prj", name="prj" + nm)

                def mk_mm(j):
                    def go():
                        nc.tensor.matmul(
                            ps[:], w_t[:, j, :], xt_tiles[s][:, j, :],
                            start=(j == 0), stop=(j == 7),
                        )

                    return go

                mms = [mk_mm(j) for j in range(8)]
                raw = rawp.tile([128, 512], bf16, tag="raw" + nm, name="raw" + nm)

                def copy_raw():
                    emit_copy(raw[:], ps[:], 512)

                swp = prjp.tile([128, 512], f32, tag="prj", name="swp" + nm)

                def do_swap():
                    nc.tensor.matmul(swp[:], PT_t[:], raw[:], start=True, stop=True)

                t1 = ropep.tile([128, 512], bf16, tag="t1" + nm)
                t2 = ropep.tile([128, 512], bf16, tag="t2" + nm)

                def r1():
                    nc.gpsimd.tensor_tensor(t1[:], raw[:], cos_sb[:, lcol], OP.mult)

                def r2():
                    nc.vector.tensor_tensor(t2[:], swp[:], sin_sb[:, lcol], OP.mult)
                    bal.charge("dve", 512)

                def r3():
                    nc.vector.tensor_tensor(dest[:, dcol], t1[:], t2[:], OP.add)
                    bal.charge("dve", 256)  # bf16 2x mode

                return mms, copy_raw, do_swap, [r1, r2, r3]

            kmm, kcopy, kswap, krope = qk_closures("k", wk_t, kT)
            qmm, qcopy, qswap, qrope = qk_closures("q", wq_t, qT)

            # V^T: out [128 t, 128 d] per 128-token block, lhsT = x chunk
            pv = vopp.tile([128, 4, 128], f32, tag="vop", name="pv")

            vcls = []
            for tblk in range(4):
                for jh in range(2):

                    def go(tblk=tblk, jh=jh):
                        for j in range(jh * 4, jh * 4 + 4):
                            nc.tensor.matmul(
                                pv[:, tblk, :],
                                xt_tiles[s][:, j, ds(tblk * 128, 128)],
                                wv_t[:, j, :],
                                start=(j == 0), stop=(j == 7),
                            )

                    vcls.append(go)

            def mk_vcopy(h):
                def go():
                    blk0 = (h * 2 + b) * 16 + ci * 4
                    emit_copy(
                        v_sb[:, ds(blk0, 4), 0:64],
                        pv[:, :, ds(h * 64, 64)],
                        256,
                    )

                return go

            cls.extend(kmm)
            cls.append(kcopy)
            cls.extend(vcls[0:2])
            cls.append(kswap)
            cls.append(krope[0])
            cls.append(krope[1])
            cls.append(krope[2])
            cls.extend(qmm)
            cls.append(qcopy)
            cls.extend(vcls[2:4])
            cls.append(qswap)
            cls.append(qrope[0])
            cls.append(qrope[1])
            cls.append(qrope[2])
            cls.extend(vcls[4:8])
            cls.append(mk_vcopy(0))
            cls.append(mk_vcopy(1))
            return cls

        # ------------------------------------------------ output projection
        def make_oproj(b, qi):
            qcol0 = b * 2048 + qi * 512
            ob = obp.tile([128, 4, 1024], bf16, tag="ob", name="ob")
            state = {"n": 0}

            def piece(tcki, oc):
                def go():
                    op = vopp.tile([128, 512], f32, tag="vop", name="op")
                    nc.tensor.matmul(
                        op[:],
                        y2[:, ds(qcol0 + tcki * 128, 128)],
                        woC[:, ds(oc * 512, 512)],
                        start=True, stop=True,
                    )
                    emit_copy(ob[:, tcki, ds(oc * 512, 512)], op[:], 512)
                    state["n"] += 1
                    if state["n"] == 8:
                        dst = (
                            d["outp"][ds(qcol0, 512), :]
                            .rearrange("(t p) o -> p t o", t=4)
                        )
                        nc.sync.dma_start(dst, ob[:])

                return go

            return [piece(t, o) for t in range(4) for o in range(2)]

        # ------------------------------------------------ epilogue (normalize)
        def make_epilogue(b, qi, h, yp, c0=0, w=512, merge=True):
            qcols = ds(b * 2048 + qi * 512 + c0, w)
            cw = ds(c0, w)
            st = {}

            def e1():
                rc = rcp.tile([1, 512], bf16, tag="rc%d" % h, name="rc")
                nc.vector.reciprocal(rc[:, 0:w], yp[64:65, cw])
                bal.charge("dve", w)
                st["rc"] = rc

            def e2():
                rbq = spp.tile([128, 512], f32, tag="sp", name="rbq")
                nc.tensor.matmul(
                    rbq[0:64, 0:w], ones1[:], st["rc"][:, 0:w],
                    start=True, stop=True,
                )
                st["rbq"] = rbq

            def e3():
                rb = rbp.tile([64, 512], bf16, tag="rb%d" % h, name="rb")
                # pin to ACT: recip (DVE) -> rbq (PE) -> rb (ACT) -> norm (DVE)
                # pipelines across engines instead of serializing on DVE
                bal.charge("act", w)
                nc.scalar.copy(rb[:, 0:w], st["rbq"][0:64, 0:w])
                st["rb"] = rb

            def e4():
                dst = y2[0:64, qcols] if h == 0 else y2B[0:64, qcols]
                nc.vector.tensor_tensor(
                    dst, yp[0:64, cw], st["rb"][:, 0:w], OP.mult
                )
                bal.charge("dve", w)
                if h == 1 and merge:
                    nc.sync.dma_start(y2[64:128, qcols], y2B[0:64, qcols])

            return [e1, e2, e3, e4]

        # ------------------------------------------------ attention section
        def section(b, qi, last=False):
            qcol0 = b * 2048 + qi * 512
            nj = 4 * qi + 4
            n_slots = 2 * (2 * nj + PIPE)

            def pump_ratio():
                # front-loaded: drain filler in the first ~60% of the section
                # so next-slab RoPE chains complete before the next section
                k = len(filler)
                slots = max(1, (st_slots[0] * 3) // 5)
                return max(1, -(-k // slots)) if k else 0

            st_slots = [n_slots]
            for h in range(2):
                yp = ypp.tile([65, 512], f32, tag="yp%d" % h, name="yp")
                # Diagonal (masked) tiles early so their exp+mask chain hides
                # mid-pipeline, but after two full tiles (old slabs) so the
                # section start doesn't wait on the newest slab's RoPE chain.
                fulls = list(range(0, nj - 4))
                diags = list(range(nj - 1, nj - 5, -1))
                cut = min(4, len(fulls))
                js = fulls[0:cut] + diags + fulls[cut:]
                inflight = []
                n_popped = [0]
                for idx in range(nj + PIPE):
                    if idx < nj:
                        j = js[idx]
                        dlt = j * 128 - qi * 512
                        dlt0 = max(dlt, 0)
                        w = 512 - dlt0
                        sp = spp.tile([128, 512], f32, tag="sp", name="sp")
                        nc.tensor.matmul(
                            sp[:, ds(dlt0, w)],
                            kT[64 * h : 64 * h + 64, ds(b * 2048 + j * 128, 128)],
                            qT[64 * h : 64 * h + 64, ds(qcol0 + dlt0, w)],
                            start=True, stop=True,
                        )
                        inflight.append((j, dlt, dlt0, sp))
                        st_slots[0] -= 1
                        pump(pump_ratio())
                    if len(inflight) >= PIPE or (idx >= nj and inflight):
                        j, dlt, dlt0, sp = inflight.pop(0)
                        w = 512 - dlt0
                        E = ep.tile([128, 512], bf16, tag="E", name="E")
                        nc.scalar.activation(
                            E[:, ds(dlt0, w)], sp[:, ds(dlt0, w)], AF.Exp
                        )
                        bal.charge("act", w)
                        if dlt >= 0:
                            nc.gpsimd.tensor_tensor(
                                E[:, ds(dlt, 128)],
                                E[:, ds(dlt, 128)],
                                tri_t[:],
                                OP.mult,
                            )
                        blk = (h * 2 + b) * 16 + j
                        nc.tensor.matmul(
                            yp[:, ds(dlt0, w)],
                            v_sb[:, blk, :],
                            E[:, ds(dlt0, w)],
                            start=(n_popped[0] == 0), stop=(n_popped[0] == nj - 1),
                            skip_group_check=True,
                        )
                        n_popped[0] += 1
                        st_slots[0] -= 1
                        pump(pump_ratio())
                if last:
                    if h == 0:
                        filler.extend(make_epilogue(b, qi, 0, yp, merge=False))
                    else:
                        ep_halves[0] = make_epilogue(b, qi, 1, yp, 0, 256, False)
                        ep_halves[1] = make_epilogue(b, qi, 1, yp, 256, 256, False)
                else:
                    filler.extend(make_epilogue(b, qi, h, yp))

        # ------------------------------------------------ schedule
        # startup: project slab (0,0) directly
        for c in make_proj(0, 0):
            c()

        ep_halves = {}
        chunks = [(b, qi) for b in range(2) for qi in range(4)]
        for sidx, (b, qi) in enumerate(chunks):
            # enqueue filler: next slab's projections + output projection of
            # the chunk two sections back (delayed so tail sections get work)
            if qi < 3:
                filler.extend(make_proj(b, qi + 1))
            elif b == 0:
                filler.extend(make_proj(1, 0))
            for oidx in [sidx - 2] + ([6] if sidx == 7 else []):
                if 0 <= oidx:
                    filler.extend([lambda: None] * 4)
                    filler.extend(make_oproj(*chunks[oidx]))
            section(b, qi, last=(b, qi) == (1, 3))
        while filler:
            pump()

        # final chunk: both piecewise h1 epilogue chains launched up-front so
        # they pipeline across DVE/PE/ACT, then split-contraction O-proj
        # pieces (head halves contracted separately; no y2 merge DMA needed),
        # copies alternating ACT/DVE, DMA fired per 128-token block
        qcol0 = 1 * 2048 + 3 * 512
        obL = obp.tile([128, 4, 1024], bf16, tag="ob", name="obL")
        chains = [list(ep_halves[0]), list(ep_halves[1])]
        for step in range(4):
            for half in range(2):
                chains[half][step]()
        for tcki in range(4):
            for oc in range(2):
                op = vopp.tile([128, 512], f32, tag="vop", name="opL")
                nc.tensor.matmul(
                    op[:],
                    y2[0:64, ds(qcol0 + tcki * 128, 128)],
                    woC[0:64, ds(oc * 512, 512)],
                    start=True, stop=False,
                )
                nc.tensor.matmul(
                    op[:],
                    y2B[0:64, ds(qcol0 + tcki * 128, 128)],
                    woC_lo[:, ds(oc * 512, 512)],
                    start=False, stop=True,
                )
                eng = "act" if oc == 0 else "dve"
                bal.charge(eng, 512)
                if eng == "act":
                    nc.scalar.copy(obL[:, tcki, ds(oc * 512, 512)], op[:])
                else:
                    nc.vector.tensor_copy(obL[:, tcki, ds(oc * 512, 512)], op[:])
            dst = (
                d["outp"][ds(qcol0 + tcki * 128, 128), :]
                .rearrange("(t p) o -> p t o", t=1)
            )
            nc.sync.dma_start(dst, obL[:, ds(tcki, 1), :])


_NC_CACHE = {}


def _build():
    if "nc" in _NC_CACHE:
        return _NC_CACHE["nc"]
    import concourse.bass as bass
    import concourse.mybir as mybir
    import concourse.tile as tile

    f32 = mybir.dt.float32
    bf16 = mybir.dt.bfloat16
    nc = bass.Bass("TRN2", target_bir_lowering=False, debug=False, num_devices=1)
    d = {
        "xt": nc.dram_tensor("xt", [8, 128, 8, 512], bf16, kind="ExternalInput").ap(),
        "wqT": nc.dram_tensor("wqT", [128, 8, 128], bf16, kind="ExternalInput").ap(),
        "wkT": nc.dram_tensor("wkT", [128, 8, 128], bf16, kind="ExternalInput").ap(),
        "wvT": nc.dram_tensor("wvT", [128, 8, 128], bf16, kind="ExternalInput").ap(),
        "woC": nc.dram_tensor("woC", [128, 1024], bf16, kind="ExternalInput").ap(),
        "cos_t": nc.dram_tensor("cos_t", [128, 2048], bf16, kind="ExternalInput").ap(),
        "sin_t": nc.dram_tensor("sin_t", [128, 2048], bf16, kind="ExternalInput").ap(),
        "tri": nc.dram_tensor("tri", [128, 128], bf16, kind="ExternalInput").ap(),
        "PT": nc.dram_tensor("PT", [128, 128], bf16, kind="ExternalInput").ap(),
        "ones1": nc.dram_tensor("ones1", [1, 64], bf16, kind="ExternalInput").ap(),
        "outp": nc.dram_tensor("outp", [4096, 1024], bf16, kind="ExternalOutput").ap(),
    }
    with tile.TileContext(nc) as tc:
        _emit(nc, tc, d)
    _split_excess_waits(nc)
    _NC_CACHE["nc"] = nc
    return nc


def kernel(x, wq, wk, wv, wo, rope_cos, rope_sin):
    from concourse import bass_utils

    x, wq, wk, wv, wo, rope_cos, rope_sin = (
        np.asarray(a, dtype=np.float32)
        for a in (x, wq, wk, wv, wo, rope_cos, rope_sin)
    )
    in_maps = _make_core_inputs(x, wq, wk, wv, wo, rope_cos, rope_sin)
    nc = _build()
    res = bass_utils.run_bass_kernel_spmd(nc, in_maps, core_ids=list(range(N_CORES)))
    total = np.zeros((B * T, C), np.float32)
    for i in range(N_CORES):
        total += res.results[i]["outp"].astype(np.float32)
    return total.reshape(B, T, C).astype(np.float32)


# revision 30
# speedup vs baseline: 1.0374x; 1.0374x over previous
"""Trainium2 Bass kernel for nn_CausalSelfAttention (BitLinear QKV/O + RoPE + causal attn).

Sharding: head-parallel, 2 heads x 2 batches per core; all matmul operands bf16
(fp32 PSUM accum). Single fused software pipeline: each attention chunk's
j-loop (scores -> exp -> E@V) is interleaved with "filler" work -- the next
slab's Q/K/V projections + RoPE and the previous chunk's output projection --
so the PE stream never drains. V is produced pre-transposed ([token, dim]) by
restructuring its projection (x-chunk as lhsT), eliminating PE transposes.
Diagonal (masked) tiles run first in reversed-j order so their exp+mask chain
hides under the pipeline fill. Q->K share one PSUM bank sequentially; scores
use a 3-deep PSUM rotation; exp runs on ACT; PSUM evacuation copies are
balanced ACT/DVE; RoPE cos/sin multiplies run on Pool. Partial outputs are
written bf16 and summed across cores on the host.
"""
import sys

sys.path.insert(0, "/opt/trn_rl_repo")

from collections import deque

import numpy as np

GROUP = 128
N_HEADS = 16
EPS = 1e-8
B, T, C = 2, 2048, 1024
HD = 64
N_CORES = 8
HPC = N_HEADS // N_CORES  # 2 heads per core
PIPE = 3


# ---------------------------------------------------------------- host prep
def _ternary_quantize(w):
    O, I = w.shape
    g = w.reshape(O, I // GROUP, GROUP).astype(np.float32)
    scale = np.maximum(np.mean(np.abs(g), axis=-1, keepdims=True), EPS).astype(
        np.float32
    )
    wn = g / scale
    q = np.where(wn > 0.5, 1.0, np.where(wn < -0.5, -1.0, 0.0)).astype(np.float32)
    return (q * scale).reshape(O, I).astype(np.float32)


def _np_bf16():
    import concourse.mybir as mybir

    return np.dtype(mybir.dt.np(mybir.dt.bfloat16))


def _make_core_inputs(x, wq, wk, wv, wo, rope_cos, rope_sin):
    """Returns list of 8 per-core input dicts (bf16 device layouts)."""
    bf = _np_bf16()
    x = np.ascontiguousarray(x.astype(np.float32).reshape(B * T, C))
    wq_q = _ternary_quantize(wq) * np.float32(HD**-0.5)  # fold attn scale
    wk_q = _ternary_quantize(wk)
    wv_q = _ternary_quantize(wv)
    wo_q = _ternary_quantize(wo)

    xT = x.T  # [1024 c, 4096 t]
    xt_slab = np.ascontiguousarray(
        xT.reshape(8, 128, 8, 512).transpose(2, 1, 0, 3)
    ).astype(bf)  # [s, p, cc, u]

    cosT = rope_cos.astype(np.float32).T  # [32, 2048]
    sinT = rope_sin.astype(np.float32).T
    cos_t = np.tile(cosT, (4, 1)).astype(bf)
    sin_t = np.concatenate([-sinT, sinT, -sinT, sinT], axis=0).astype(bf)
    tri = (np.arange(128)[None, :] >= np.arange(128)[:, None]).astype(bf)
    # partition-swap matrix: out = PT^T @ in, out[m] = in[sigma(m)],
    # sigma swaps 32-row halves within each 64-row head block.
    m = np.arange(128)
    sigma = np.where(m % 64 < 32, m + 32, m - 32)
    PT = np.zeros((128, 128), np.float32)
    PT[sigma, m] = 1.0
    PT = PT.astype(bf)

    maps = []
    for core in range(N_CORES):
        r0 = core * HPC * HD
        rows = slice(r0, r0 + HPC * HD)

        def w_lhsT(w_qq):
            wsT = w_qq[rows, :].T  # [1024 in, 128 d]
            return np.ascontiguousarray(
                wsT.reshape(8, 128, 128).transpose(1, 0, 2)
            ).astype(bf)

        maps.append(
            {
                "xt": xt_slab,
                "wqT": w_lhsT(wq_q),
                "wkT": w_lhsT(wk_q),
                "wvT": w_lhsT(wv_q),
                "woC": np.ascontiguousarray(wo_q[:, rows].T).astype(bf),
                "cos_t": cos_t,
                "sin_t": sin_t,
                "tri": tri,
                "PT": PT,
                "ones1": np.ones((1, 64), np.float32).astype(bf),
            }
        )
    return maps


# ---------------------------------------------------------------- BIR post-pass
def _split_excess_waits(nc, max_waits=1):
    """walrus CoreV3 codegen rejects instructions with >1 sem wait; split the
    excess into preceding NoOps on the same engine."""
    import concourse.mybir as mybir

    for f in nc.m.functions:
        for bb in f.blocks:
            insts = bb.instructions
            i = 0
            while i < len(insts):
                ins = insts[i]
                si = ins.sync_info
                if si is not None and si.on_wait and len(si.on_wait) > max_waits:
                    waits = list(si.on_wait)
                    si.on_wait = waits[:max_waits]
                    rest = waits[max_waits:]
                    new_ops = []
                    for j in range(0, len(rest), max_waits):
                        new_ops.append(
                            mybir.InstNoOp(
                                name=nc.get_next_instruction_name(),
                                sync_info=mybir.SyncInfo(
                                    on_wait=rest[j : j + max_waits], on_update=[]
                                ),
                                bass_nofuse=True,
                                engine=ins.engine,
                            )
                        )
                    insts[i:i] = new_ops
                    i += len(new_ops)
                i += 1


# ---------------------------------------------------------------- device kernel
class _Balancer:
    """Greedy least-loaded assignment of PSUM-evacuation copies to ACT/DVE."""

    def __init__(self):
        self.busy = {"act": 0.0, "dve": 0.0}

    @staticmethod
    def cost(eng, free):
        if eng == "act":
            return free * 0.833 + 185.0
        return free * 1.042 + 125.0

    def charge(self, eng, free):
        self.busy[eng] += self.cost(eng, free)
        return eng

    def pick(self, free, bias=None):
        bias = bias or {}
        best = min(
            ("act", "dve"),
            key=lambda e: self.busy[e] + self.cost(e, free) + bias.get(e, 0.0),
        )
        return self.charge(best, free)


def _emit(nc, tc, d):
    import concourse.mybir as mybir
    from concourse.bass import ds

    f32 = mybir.dt.float32
    bf16 = mybir.dt.bfloat16
    AF = mybir.ActivationFunctionType
    OP = mybir.AluOpType
    bal = _Balancer()

    def emit_copy(dst, src, free, bias=None):
        eng = bal.pick(free, bias)
        if eng == "act":
            nc.scalar.copy(dst, src)
        else:
            nc.vector.tensor_copy(dst, src)

    with nc.allow_low_precision(
        reason="bf16 matmul operands; fp32 accum in PSUM"
    ), tc.tile_pool(name="const", bufs=1) as cp, tc.tile_pool(
        name="persist", bufs=1
    ) as pp, tc.tile_pool(name="xt", bufs=3) as xtp, tc.tile_pool(
        name="raw", bufs=2
    ) as rawp, tc.tile_pool(name="rope", bufs=2) as ropep, tc.tile_pool(
        name="E", bufs=6
    ) as ep, tc.tile_pool(name="rc", bufs=2) as rcp, tc.tile_pool(
        name="rb", bufs=2
    ) as rbp, tc.tile_pool(name="ob", bufs=2) as obp, tc.tile_pool(
        name="prj", bufs=1, space="PSUM"
    ) as prjp, tc.tile_pool(name="vop", bufs=2, space="PSUM") as vopp, tc.tile_pool(
        name="yp", bufs=1, space="PSUM"
    ) as ypp, tc.tile_pool(name="sp", bufs=3, space="PSUM") as spp:
        # ---- constants / inputs: K weights + x first (K projects first),
        # spread across the SP/ACT/DVE DMA rings so issue overhead overlaps
        wk_t = cp.tile([128, 8, 128], bf16)
        nc.sync.dma_start(wk_t[:, 0:2, :], d["wkT"][:, 0:2, :])
        xt0 = xtp.tile([128, 8, 512], bf16, tag="xt", name="xt0")
        nc.sync.dma_start(xt0[:, 0:2, :], d["xt"][0][:, 0:2, :])
        nc.sync.dma_start(wk_t[:, 2:8, :], d["wkT"][:, 2:8, :])
        nc.sync.dma_start(xt0[:, 2:5, :], d["xt"][0][:, 2:5, :])
        wq_t = cp.tile([128, 8, 128], bf16)
        nc.scalar.dma_start(wq_t[:], d["wqT"])
        wv_t = cp.tile([128, 8, 128], bf16)
        nc.scalar.dma_start(wv_t[:], d["wvT"])
        nc.sync.dma_start(xt0[:, 5:8, :], d["xt"][0][:, 5:8, :])
        xt1 = xtp.tile([128, 8, 512], bf16, tag="xt", name="xt1")
        nc.sync.dma_start(xt1[:], d["xt"][1])
        PT_t = cp.tile([128, 128], bf16)
        nc.scalar.dma_start(PT_t[:], d["PT"])
        cos_sb = cp.tile([128, 2048], bf16)
        nc.scalar.dma_start(cos_sb[:], d["cos_t"])
        sin_sb = cp.tile([128, 2048], bf16)
        nc.scalar.dma_start(sin_sb[:], d["sin_t"])
        tri_t = cp.tile([128, 128], bf16)
        nc.scalar.dma_start(tri_t[:], d["tri"])
        woC = cp.tile([128, 1024], bf16)
        nc.scalar.dma_start(woC[:], d["woC"])
        woC_lo = cp.tile([64, 1024], bf16)
        nc.scalar.dma_start(woC_lo[:], d["woC"][64:128, :])
        ones1 = cp.tile([1, 64], bf16)
        nc.scalar.dma_start(ones1[:], d["ones1"])

        qT = pp.tile([128, 4096], bf16)
        kT = pp.tile([128, 4096], bf16)
        v_sb = pp.tile([128, 64, 65], bf16)
        nc.gpsimd.memset(v_sb[:, :, 64:65], 1.0)  # denominator ones column
        y2 = pp.tile([128, 4096], bf16)
        y2B = pp.tile([64, 4096], bf16)

        xt_tiles = {0: xt0, 1: xt1}

        def load_slab(s):
            t = xtp.tile([128, 8, 512], bf16, tag="xt", name="xt%d" % s)
            nc.sync.dma_start(t[:], d["xt"][s])
            xt_tiles[s] = t

        filler = deque()

        def pump(k=1):
            for _ in range(k):
                if filler:
                    filler.popleft()()

        # ------------------------------------------------ projection closures
        def make_proj(b, ci):
            """Closure list: K then Q proj (shared PSUM tag, sequential), V^T
            proj, RoPE. Emitted as filler inside the previous attention
            section. K first so its longer RoPE chain hides under Q/V mms."""
            s = b * 4 + ci
            cls = []
            if s + 2 < 8:
                cls.append(lambda s2=s + 2: load_slab(s2))

            dcol = ds(s * 512, 512)
            lcol = ds(ci * 512, 512)

            def qk_closures(nm, w_t, dest):
                ps = prjp.tile([128, 512], f32, tag="prj", name="prj" + nm)

                def mk_mm(j):
                    def go():
                        nc.tensor.matmul(
                            ps[:], w_t[:, j, :], xt_tiles[s][:, j, :],
                            start=(j == 0), stop=(j == 7),
                        )

                    return go

                mms = [mk_mm(j) for j in range(8)]
                raw = rawp.tile([128, 512], bf16, tag="raw" + nm, name="raw" + nm)

                def copy_raw():
                    emit_copy(raw[:], ps[:], 512)

                swp = prjp.tile([128, 512], f32, tag="prj", name="swp" + nm)

                def do_swap():
                    nc.tensor.matmul(swp[:], PT_t[:], raw[:], start=True, stop=True)

                t1 = ropep.tile([128, 512], bf16, tag="t1" + nm)
                t2 = ropep.tile([128, 512], bf16, tag="t2" + nm)

                def r1():
                    nc.gpsimd.tensor_tensor(t1[:], raw[:], cos_sb[:, lcol], OP.mult)

                def r2():
                    nc.vector.tensor_tensor(t2[:], swp[:], sin_sb[:, lcol], OP.mult)
                    bal.charge("dve", 512)

                def r3():
                    nc.vector.tensor_tensor(dest[:, dcol], t1[:], t2[:], OP.add)
                    bal.charge("dve", 256)  # bf16 2x mode

                return mms, copy_raw, do_swap, [r1, r2, r3]

            kmm, kcopy, kswap, krope = qk_closures("k", wk_t, kT)
            qmm, qcopy, qswap, qrope = qk_closures("q", wq_t, qT)

            # V^T: out [128 t, 128 d] per 128-token block, lhsT = x chunk
            pv = vopp.tile([128, 4, 128], f32, tag="vop", name="pv")

            vcls = []
            for tblk in range(4):
                for jh in range(2):

                    def go(tblk=tblk, jh=jh):
                        for j in range(jh * 4, jh * 4 + 4):
                            nc.tensor.matmul(
                                pv[:, tblk, :],
                                xt_tiles[s][:, j, ds(tblk * 128, 128)],
                                wv_t[:, j, :],
                                start=(j == 0), stop=(j == 7),
                            )

                    vcls.append(go)

            def mk_vcopy(h):
                def go():
                    blk0 = (h * 2 + b) * 16 + ci * 4
                    emit_copy(
                        v_sb[:, ds(blk0, 4), 0:64],
                        pv[:, :, ds(h * 64, 64)],
                        256,
                    )

                return go

            cls.extend(kmm)
            cls.append(kcopy)
            cls.extend(vcls[0:2])
            cls.append(kswap)
            cls.append(krope[0])
            cls.append(krope[1])
            cls.append(krope[2])
            cls.extend(qmm)
            cls.append(qcopy)
            cls.extend(vcls[2:4])
            cls.append(qswap)
            cls.append(qrope[0])
            cls.append(qrope[1])
            cls.append(qrope[2])
            cls.extend(vcls[4:8])
            cls.append(mk_vcopy(0))
            cls.append(mk_vcopy(1))
            return cls

        # ------------------------------------------------ output projection
        def make_oproj(b, qi):
            qcol0 = b * 2048 + qi * 512
            ob = obp.tile([128, 4, 1024], bf16, tag="ob", name="ob")
            state = {"n": 0}

            def piece(tcki, oc):
                def go():
                    op = vopp.tile([128, 512], f32, tag="vop", name="op")
                    nc.tensor.matmul(
                        op[:],
                        y2[:, ds(qcol0 + tcki * 128, 128)],
                        woC[:, ds(oc * 512, 512)],
                        start=True, stop=True,
                    )
                    emit_copy(ob[:, tcki, ds(oc * 512, 512)], op[:], 512)
                    state["n"] += 1
                    if state["n"] == 8:
                        dst = (
                            d["outp"][ds(qcol0, 512), :]
                            .rearrange("(t p) o -> p t o", t=4)
                        )
                        nc.sync.dma_start(dst, ob[:])

                return go

            return [piece(t, o) for t in range(4) for o in range(2)]

        # ------------------------------------------------ epilogue (normalize)
        def make_epilogue(b, qi, h, yp, c0=0, w=512, merge=True):
            qcols = ds(b * 2048 + qi * 512 + c0, w)
            cw = ds(c0, w)
            st = {}

            def e1():
                rc = rcp.tile([1, 512], bf16, tag="rc%d" % h, name="rc")
                nc.vector.reciprocal(rc[:, 0:w], yp[64:65, cw])
                bal.charge("dve", w)
                st["rc"] = rc

            def e2():
                rbq = spp.tile([128, 512], f32, tag="sp", name="rbq")
                nc.tensor.matmul(
                    rbq[0:64, 0:w], ones1[:], st["rc"][:, 0:w],
                    start=True, stop=True,
                )
                st["rbq"] = rbq

            def e3():
                rb = rbp.tile([64, 512], bf16, tag="rb%d" % h, name="rb")
                # pin to ACT: recip (DVE) -> rbq (PE) -> rb (ACT) -> norm (DVE)
                # pipelines across engines instead of serializing on DVE
                bal.charge("act", w)
                nc.scalar.copy(rb[:, 0:w], st["rbq"][0:64, 0:w])
                st["rb"] = rb

            def e4():
                dst = y2[0:64, qcols] if h == 0 else y2B[0:64, qcols]
                nc.vector.tensor_tensor(
                    dst, yp[0:64, cw], st["rb"][:, 0:w], OP.mult
                )
                bal.charge("dve", w)
                if h == 1 and merge:
                    nc.sync.dma_start(y2[64:128, qcols], y2B[0:64, qcols])

            return [e1, e2, e3, e4]

        # ------------------------------------------------ attention section
        def section(b, qi, last=False):
            qcol0 = b * 2048 + qi * 512
            nj = 4 * qi + 4
            n_slots = 2 * (2 * nj + PIPE)

            def pump_ratio():
                # front-loaded: drain filler in the first ~60% of the section
                # so next-slab RoPE chains complete before the next section
                k = len(filler)
                slots = max(1, (st_slots[0] * 3) // 5)
                return max(1, -(-k // slots)) if k else 0

            st_slots = [n_slots]
            for h in range(2):
                yp = ypp.tile([65, 512], f32, tag="yp%d" % h, name="yp")
                # Diagonal (masked) tiles early so their exp+mask chain hides
                # mid-pipeline, but after two full tiles (old slabs) so the
                # section start doesn't wait on the newest slab's RoPE chain.
                fulls = list(range(0, nj - 4))
                diags = list(range(nj - 1, nj - 5, -1))
                cut = min(4, len(fulls))
                js = fulls[0:cut] + diags + fulls[cut:]
                inflight = []
                n_popped = [0]
                for idx in range(nj + PIPE):
                    if idx < nj:
                        j = js[idx]
                        dlt = j * 128 - qi * 512
                        dlt0 = max(dlt, 0)
                        w = 512 - dlt0
                        sp = spp.tile([128, 512], f32, tag="sp", name="sp")
                        nc.tensor.matmul(
                            sp[:, ds(dlt0, w)],
                            kT[64 * h : 64 * h + 64, ds(b * 2048 + j * 128, 128)],
                            qT[64 * h : 64 * h + 64, ds(qcol0 + dlt0, w)],
                            start=True, stop=True,
                        )
                        inflight.append((j, dlt, dlt0, sp))
                        st_slots[0] -= 1
                        pump(pump_ratio())
                    if len(inflight) >= PIPE or (idx >= nj and inflight):
                        j, dlt, dlt0, sp = inflight.pop(0)
                        w = 512 - dlt0
                        E = ep.tile([128, 512], bf16, tag="E", name="E")
                        nc.scalar.activation(
                            E[:, ds(dlt0, w)], sp[:, ds(dlt0, w)], AF.Exp
                        )
                        bal.charge("act", w)
                        if dlt >= 0:
                            nc.vector.tensor_tensor(
                                E[:, ds(dlt, 128)],
                                E[:, ds(dlt, 128)],
                                tri_t[:],
                                OP.mult,
                            )
                            bal.charge("dve", 128)
                        blk = (h * 2 + b) * 16 + j
                        nc.tensor.matmul(
                            yp[:, ds(dlt0, w)],
                            v_sb[:, blk, :],
                            E[:, ds(dlt0, w)],
                            start=(n_popped[0] == 0), stop=(n_popped[0] == nj - 1),
                            skip_group_check=True,
                        )
                        n_popped[0] += 1
                        st_slots[0] -= 1
                        pump(pump_ratio())
                if last:
                    if h == 0:
                        filler.extend(make_epilogue(b, qi, 0, yp, merge=False))
                    else:
                        ep_halves[0] = make_epilogue(b, qi, 1, yp, 0, 256, False)
                        ep_halves[1] = make_epilogue(b, qi, 1, yp, 256, 256, False)
                else:
                    filler.extend(make_epilogue(b, qi, h, yp))

        # ------------------------------------------------ schedule
        # startup: project slab (0,0) directly
        for c in make_proj(0, 0):
            c()

        ep_halves = {}
        chunks = [(b, qi) for b in range(2) for qi in range(4)]
        for sidx, (b, qi) in enumerate(chunks):
            # enqueue filler: next slab's projections + output projection of
            # the chunk two sections back (delayed so tail sections get work)
            if qi < 3:
                filler.extend(make_proj(b, qi + 1))
            elif b == 0:
                filler.extend(make_proj(1, 0))
            for oidx in [sidx - 2] + ([6] if sidx == 7 else []):
                if 0 <= oidx:
                    filler.extend([lambda: None] * 4)
                    filler.extend(make_oproj(*chunks[oidx]))
            section(b, qi, last=(b, qi) == (1, 3))
        while filler:
            pump()

        # final chunk: both piecewise h1 epilogue chains launched up-front so
        # they pipeline across DVE/PE/ACT, then split-contraction O-proj
        # pieces (head halves contracted separately; no y2 merge DMA needed),
        # copies alternating ACT/DVE, DMA fired per 128-token block
        qcol0 = 1 * 2048 + 3 * 512
        obL = obp.tile([128, 4, 1024], bf16, tag="ob", name="obL")
        chains = [list(ep_halves[0]), list(ep_halves[1])]
        for step in range(4):
            for half in range(2):
                chains[half][step]()
        for tcki in range(4):
            for oc in range(2):
                op = vopp.tile([128, 512], f32, tag="vop", name="opL")
                nc.tensor.matmul(
                    op[:],
                    y2[0:64, ds(qcol0 + tcki * 128, 128)],
                    woC[0:64, ds(oc * 512, 512)],
                    start=True, stop=False,
                )
                nc.tensor.matmul(
                    op[:],
                    y2B[0:64, ds(qcol0 + tcki * 128, 128)],
                    woC_lo[:, ds(oc * 512, 512)],
                    start=False, stop=True,
                )
                eng = "act" if oc == 0 else "dve"
                bal.charge(eng, 512)
                if eng == "act":
                    nc.scalar.copy(obL[:, tcki, ds(oc * 512, 512)], op[:])
                else:
                    nc.vector.tensor_copy(obL[:, tcki, ds(oc * 512, 512)], op[:])
            dst = (
                d["outp"][ds(qcol0 + tcki * 128, 128), :]
                .rearrange("(t p) o -> p t o", t=1)
            )
            nc.sync.dma_start(dst, obL[:, ds(tcki, 1), :])


_NC_CACHE = {}


def _build():
    if "nc" in _NC_CACHE:
        return _NC_CACHE["nc"]
    import concourse.bass as bass
    import concourse.mybir as mybir
    import concourse.tile as tile

    f32 = mybir.dt.float32
    bf16 = mybir.dt.bfloat16
    nc = bass.Bass("TRN2", target_bir_lowering=False, debug=False, num_devices=1)
    d = {
        "xt": nc.dram_tensor("xt", [8, 128, 8, 512], bf16, kind="ExternalInput").ap(),
        "wqT": nc.dram_tensor("wqT", [128, 8, 128], bf16, kind="ExternalInput").ap(),
        "wkT": nc.dram_tensor("wkT", [128, 8, 128], bf16, kind="ExternalInput").ap(),
        "wvT": nc.dram_tensor("wvT", [128, 8, 128], bf16, kind="ExternalInput").ap(),
        "woC": nc.dram_tensor("woC", [128, 1024], bf16, kind="ExternalInput").ap(),
        "cos_t": nc.dram_tensor("cos_t", [128, 2048], bf16, kind="ExternalInput").ap(),
        "sin_t": nc.dram_tensor("sin_t", [128, 2048], bf16, kind="ExternalInput").ap(),
        "tri": nc.dram_tensor("tri", [128, 128], bf16, kind="ExternalInput").ap(),
        "PT": nc.dram_tensor("PT", [128, 128], bf16, kind="ExternalInput").ap(),
        "ones1": nc.dram_tensor("ones1", [1, 64], bf16, kind="ExternalInput").ap(),
        "outp": nc.dram_tensor("outp", [4096, 1024], bf16, kind="ExternalOutput").ap(),
    }
    with tile.TileContext(nc) as tc:
        _emit(nc, tc, d)
    _split_excess_waits(nc)
    _NC_CACHE["nc"] = nc
    return nc


def kernel(x, wq, wk, wv, wo, rope_cos, rope_sin):
    from concourse import bass_utils

    x, wq, wk, wv, wo, rope_cos, rope_sin = (
        np.asarray(a, dtype=np.float32)
        for a in (x, wq, wk, wv, wo, rope_cos, rope_sin)
    )
    in_maps = _make_core_inputs(x, wq, wk, wv, wo, rope_cos, rope_sin)
    nc = _build()
    res = bass_utils.run_bass_kernel_spmd(nc, in_maps, core_ids=list(range(N_CORES)))
    total = np.zeros((B * T, C), np.float32)
    for i in range(N_CORES):
        total += res.results[i]["outp"].astype(np.float32)
    return total.reshape(B, T, C).astype(np.float32)


# revision 31
# speedup vs baseline: 1.0447x; 1.0071x over previous
"""Trainium2 Bass kernel for nn_CausalSelfAttention (BitLinear QKV/O + RoPE + causal attn).

Sharding: head-parallel, 2 heads x 2 batches per core; all matmul operands bf16
(fp32 PSUM accum). Single fused software pipeline: each attention chunk's
j-loop (scores -> exp -> E@V) is interleaved with "filler" work -- the next
slab's Q/K/V projections + RoPE and the previous chunk's output projection --
so the PE stream never drains. V is produced pre-transposed ([token, dim]) by
restructuring its projection (x-chunk as lhsT), eliminating PE transposes.
Diagonal (masked) tiles run first in reversed-j order so their exp+mask chain
hides under the pipeline fill. Q->K share one PSUM bank sequentially; scores
use a 3-deep PSUM rotation; exp runs on ACT; PSUM evacuation copies are
balanced ACT/DVE; RoPE cos/sin multiplies run on Pool. Partial outputs are
written bf16 and summed across cores on the host.
"""
import sys

sys.path.insert(0, "/opt/trn_rl_repo")

from collections import deque

import numpy as np

GROUP = 128
N_HEADS = 16
EPS = 1e-8
B, T, C = 2, 2048, 1024
HD = 64
N_CORES = 8
HPC = N_HEADS // N_CORES  # 2 heads per core
PIPE = 3


# ---------------------------------------------------------------- host prep
def _ternary_quantize(w):
    O, I = w.shape
    g = w.reshape(O, I // GROUP, GROUP).astype(np.float32)
    scale = np.maximum(np.mean(np.abs(g), axis=-1, keepdims=True), EPS).astype(
        np.float32
    )
    wn = g / scale
    q = np.where(wn > 0.5, 1.0, np.where(wn < -0.5, -1.0, 0.0)).astype(np.float32)
    return (q * scale).reshape(O, I).astype(np.float32)


def _np_bf16():
    import concourse.mybir as mybir

    return np.dtype(mybir.dt.np(mybir.dt.bfloat16))


def _make_core_inputs(x, wq, wk, wv, wo, rope_cos, rope_sin):
    """Returns list of 8 per-core input dicts (bf16 device layouts)."""
    bf = _np_bf16()
    x = np.ascontiguousarray(x.astype(np.float32).reshape(B * T, C))
    wq_q = _ternary_quantize(wq) * np.float32(HD**-0.5)  # fold attn scale
    wk_q = _ternary_quantize(wk)
    wv_q = _ternary_quantize(wv)
    wo_q = _ternary_quantize(wo)

    xT = x.T  # [1024 c, 4096 t]
    xt_slab = np.ascontiguousarray(
        xT.reshape(8, 128, 8, 512).transpose(2, 1, 0, 3)
    ).astype(bf)  # [s, p, cc, u]

    cosT = rope_cos.astype(np.float32).T  # [32, 2048]
    sinT = rope_sin.astype(np.float32).T
    cos_t = np.tile(cosT, (4, 1)).astype(bf)
    sin_t = np.concatenate([-sinT, sinT, -sinT, sinT], axis=0).astype(bf)
    tri = (np.arange(128)[None, :] >= np.arange(128)[:, None]).astype(bf)
    # partition-swap matrix: out = PT^T @ in, out[m] = in[sigma(m)],
    # sigma swaps 32-row halves within each 64-row head block.
    m = np.arange(128)
    sigma = np.where(m % 64 < 32, m + 32, m - 32)
    PT = np.zeros((128, 128), np.float32)
    PT[sigma, m] = 1.0
    PT = PT.astype(bf)

    maps = []
    for core in range(N_CORES):
        r0 = core * HPC * HD
        rows = slice(r0, r0 + HPC * HD)

        def w_lhsT(w_qq):
            wsT = w_qq[rows, :].T  # [1024 in, 128 d]
            return np.ascontiguousarray(
                wsT.reshape(8, 128, 128).transpose(1, 0, 2)
            ).astype(bf)

        maps.append(
            {
                "xt": xt_slab,
                "wqT": w_lhsT(wq_q),
                "wkT": w_lhsT(wk_q),
                "wvT": w_lhsT(wv_q),
                "woC": np.ascontiguousarray(wo_q[:, rows].T).astype(bf),
                "cos_t": cos_t,
                "sin_t": sin_t,
                "tri": tri,
                "PT": PT,
                "ones1": np.ones((1, 64), np.float32).astype(bf),
            }
        )
    return maps


# ---------------------------------------------------------------- BIR post-pass
def _split_excess_waits(nc, max_waits=1):
    """walrus CoreV3 codegen rejects instructions with >1 sem wait; split the
    excess into preceding NoOps on the same engine."""
    import concourse.mybir as mybir

    for f in nc.m.functions:
        for bb in f.blocks:
            insts = bb.instructions
            i = 0
            while i < len(insts):
                ins = insts[i]
                si = ins.sync_info
                if si is not None and si.on_wait and len(si.on_wait) > max_waits:
                    waits = list(si.on_wait)
                    si.on_wait = waits[:max_waits]
                    rest = waits[max_waits:]
                    new_ops = []
                    for j in range(0, len(rest), max_waits):
                        new_ops.append(
                            mybir.InstNoOp(
                                name=nc.get_next_instruction_name(),
                                sync_info=mybir.SyncInfo(
                                    on_wait=rest[j : j + max_waits], on_update=[]
                                ),
                                bass_nofuse=True,
                                engine=ins.engine,
                            )
                        )
                    insts[i:i] = new_ops
                    i += len(new_ops)
                i += 1


# ---------------------------------------------------------------- device kernel
class _Balancer:
    """Greedy least-loaded assignment of PSUM-evacuation copies to ACT/DVE."""

    def __init__(self):
        self.busy = {"act": 0.0, "dve": 0.0}

    @staticmethod
    def cost(eng, free):
        if eng == "act":
            return free * 0.833 + 185.0
        return free * 1.042 + 125.0

    def charge(self, eng, free):
        self.busy[eng] += self.cost(eng, free)
        return eng

    def pick(self, free, bias=None):
        bias = bias or {}
        best = min(
            ("act", "dve"),
            key=lambda e: self.busy[e] + self.cost(e, free) + bias.get(e, 0.0),
        )
        return self.charge(best, free)


def _emit(nc, tc, d):
    import concourse.mybir as mybir
    from concourse.bass import ds

    f32 = mybir.dt.float32
    bf16 = mybir.dt.bfloat16
    AF = mybir.ActivationFunctionType
    OP = mybir.AluOpType
    bal = _Balancer()

    def emit_copy(dst, src, free, bias=None):
        eng = bal.pick(free, bias)
        if eng == "act":
            nc.scalar.copy(dst, src)
        else:
            nc.vector.tensor_copy(dst, src)

    with nc.allow_low_precision(
        reason="bf16 matmul operands; fp32 accum in PSUM"
    ), tc.tile_pool(name="const", bufs=1) as cp, tc.tile_pool(
        name="persist", bufs=1
    ) as pp, tc.tile_pool(name="xt", bufs=3) as xtp, tc.tile_pool(
        name="raw", bufs=2
    ) as rawp, tc.tile_pool(name="rope", bufs=2) as ropep, tc.tile_pool(
        name="E", bufs=6
    ) as ep, tc.tile_pool(name="rc", bufs=2) as rcp, tc.tile_pool(
        name="rb", bufs=2
    ) as rbp, tc.tile_pool(name="ob", bufs=2) as obp, tc.tile_pool(
        name="prj", bufs=1, space="PSUM"
    ) as prjp, tc.tile_pool(name="vop", bufs=2, space="PSUM") as vopp, tc.tile_pool(
        name="yp", bufs=1, space="PSUM"
    ) as ypp, tc.tile_pool(name="sp", bufs=3, space="PSUM") as spp:
        # ---- constants / inputs: K weights + x first (K projects first),
        # spread across the SP/ACT/DVE DMA rings so issue overhead overlaps
        wk_t = cp.tile([128, 8, 128], bf16)
        nc.sync.dma_start(wk_t[:, 0:2, :], d["wkT"][:, 0:2, :])
        xt0 = xtp.tile([128, 8, 512], bf16, tag="xt", name="xt0")
        nc.sync.dma_start(xt0[:, 0:2, :], d["xt"][0][:, 0:2, :])
        nc.sync.dma_start(wk_t[:, 2:8, :], d["wkT"][:, 2:8, :])
        nc.sync.dma_start(xt0[:, 2:5, :], d["xt"][0][:, 2:5, :])
        wq_t = cp.tile([128, 8, 128], bf16)
        nc.scalar.dma_start(wq_t[:], d["wqT"])
        wv_t = cp.tile([128, 8, 128], bf16)
        nc.scalar.dma_start(wv_t[:], d["wvT"])
        nc.sync.dma_start(xt0[:, 5:8, :], d["xt"][0][:, 5:8, :])
        xt1 = xtp.tile([128, 8, 512], bf16, tag="xt", name="xt1")
        nc.sync.dma_start(xt1[:, 0:4, :], d["xt"][1][:, 0:4, :])
        nc.sync.dma_start(xt1[:, 4:8, :], d["xt"][1][:, 4:8, :])
        PT_t = cp.tile([128, 128], bf16)
        nc.scalar.dma_start(PT_t[:], d["PT"])
        cos_sb = cp.tile([128, 2048], bf16)
        nc.scalar.dma_start(cos_sb[:], d["cos_t"])
        sin_sb = cp.tile([128, 2048], bf16)
        nc.scalar.dma_start(sin_sb[:], d["sin_t"])
        tri_t = cp.tile([128, 128], bf16)
        nc.scalar.dma_start(tri_t[:], d["tri"])
        woC = cp.tile([128, 1024], bf16)
        nc.sync.dma_start(woC[:], d["woC"])
        woC_lo = cp.tile([64, 1024], bf16)
        nc.sync.dma_start(woC_lo[:], d["woC"][64:128, :])
        ones1 = cp.tile([1, 64], bf16)
        nc.sync.dma_start(ones1[:], d["ones1"])

        qT = pp.tile([128, 4096], bf16)
        kT = pp.tile([128, 4096], bf16)
        v_sb = pp.tile([128, 64, 65], bf16)
        nc.gpsimd.memset(v_sb[:, :, 64:65], 1.0)  # denominator ones column
        y2 = pp.tile([128, 4096], bf16)
        y2B = pp.tile([64, 4096], bf16)

        xt_tiles = {0: xt0, 1: xt1}

        def load_slab(s):
            t = xtp.tile([128, 8, 512], bf16, tag="xt", name="xt%d" % s)
            nc.sync.dma_start(t[:], d["xt"][s])
            xt_tiles[s] = t

        filler = deque()

        def pump(k=1):
            for _ in range(k):
                if filler:
                    filler.popleft()()

        # ------------------------------------------------ projection closures
        def make_proj(b, ci):
            """Closure list: K then Q proj (shared PSUM tag, sequential), V^T
            proj, RoPE. Emitted as filler inside the previous attention
            section. K first so its longer RoPE chain hides under Q/V mms."""
            s = b * 4 + ci
            cls = []
            if s + 2 < 8:
                cls.append(lambda s2=s + 2: load_slab(s2))

            dcol = ds(s * 512, 512)
            lcol = ds(ci * 512, 512)

            def qk_closures(nm, w_t, dest):
                ps = prjp.tile([128, 512], f32, tag="prj", name="prj" + nm)

                def mk_mm(j):
                    def go():
                        nc.tensor.matmul(
                            ps[:], w_t[:, j, :], xt_tiles[s][:, j, :],
                            start=(j == 0), stop=(j == 7),
                        )

                    return go

                mms = [mk_mm(j) for j in range(8)]
                raw = rawp.tile([128, 512], bf16, tag="raw" + nm, name="raw" + nm)

                def copy_raw():
                    emit_copy(raw[:], ps[:], 512)

                swp = prjp.tile([128, 512], f32, tag="prj", name="swp" + nm)

                def do_swap():
                    nc.tensor.matmul(swp[:], PT_t[:], raw[:], start=True, stop=True)

                t1 = ropep.tile([128, 512], bf16, tag="t1" + nm)
                t2 = ropep.tile([128, 512], bf16, tag="t2" + nm)

                def r1():
                    nc.gpsimd.tensor_tensor(t1[:], raw[:], cos_sb[:, lcol], OP.mult)

                def r2():
                    nc.vector.tensor_tensor(t2[:], swp[:], sin_sb[:, lcol], OP.mult)
                    bal.charge("dve", 512)

                def r3():
                    nc.vector.tensor_tensor(dest[:, dcol], t1[:], t2[:], OP.add)
                    bal.charge("dve", 256)  # bf16 2x mode

                return mms, copy_raw, do_swap, [r1, r2, r3]

            kmm, kcopy, kswap, krope = qk_closures("k", wk_t, kT)
            qmm, qcopy, qswap, qrope = qk_closures("q", wq_t, qT)

            # V^T: out [128 t, 128 d] per 128-token block, lhsT = x chunk
            pv = vopp.tile([128, 4, 128], f32, tag="vop", name="pv")

            vcls = []
            for tblk in range(4):
                for jh in range(2):

                    def go(tblk=tblk, jh=jh):
                        for j in range(jh * 4, jh * 4 + 4):
                            nc.tensor.matmul(
                                pv[:, tblk, :],
                                xt_tiles[s][:, j, ds(tblk * 128, 128)],
                                wv_t[:, j, :],
                                start=(j == 0), stop=(j == 7),
                            )

                    vcls.append(go)

            def mk_vcopy(h):
                def go():
                    blk0 = (h * 2 + b) * 16 + ci * 4
                    emit_copy(
                        v_sb[:, ds(blk0, 4), 0:64],
                        pv[:, :, ds(h * 64, 64)],
                        256,
                    )

                return go

            cls.extend(kmm)
            cls.append(kcopy)
            cls.extend(vcls[0:2])
            cls.append(kswap)
            cls.append(krope[0])
            cls.append(krope[1])
            cls.append(krope[2])
            cls.extend(qmm)
            cls.append(qcopy)
            cls.extend(vcls[2:4])
            cls.append(qswap)
            cls.append(qrope[0])
            cls.append(qrope[1])
            cls.append(qrope[2])
            cls.extend(vcls[4:8])
            cls.append(mk_vcopy(0))
            cls.append(mk_vcopy(1))
            return cls

        # ------------------------------------------------ output projection
        def make_oproj(b, qi):
            qcol0 = b * 2048 + qi * 512
            ob = obp.tile([128, 4, 1024], bf16, tag="ob", name="ob")
            state = {"n": 0}

            def piece(tcki, oc):
                def go():
                    op = vopp.tile([128, 512], f32, tag="vop", name="op")
                    nc.tensor.matmul(
                        op[:],
                        y2[:, ds(qcol0 + tcki * 128, 128)],
                        woC[:, ds(oc * 512, 512)],
                        start=True, stop=True,
                    )
                    emit_copy(ob[:, tcki, ds(oc * 512, 512)], op[:], 512)
                    state["n"] += 1
                    if state["n"] == 8:
                        dst = (
                            d["outp"][ds(qcol0, 512), :]
                            .rearrange("(t p) o -> p t o", t=4)
                        )
                        nc.sync.dma_start(dst, ob[:])

                return go

            return [piece(t, o) for t in range(4) for o in range(2)]

        # ------------------------------------------------ epilogue (normalize)
        def make_epilogue(b, qi, h, yp, c0=0, w=512, merge=True):
            qcols = ds(b * 2048 + qi * 512 + c0, w)
            cw = ds(c0, w)
            st = {}

            def e1():
                rc = rcp.tile([1, 512], bf16, tag="rc%d" % h, name="rc")
                nc.vector.reciprocal(rc[:, 0:w], yp[64:65, cw])
                bal.charge("dve", w)
                st["rc"] = rc

            def e2():
                rbq = spp.tile([128, 512], f32, tag="sp", name="rbq")
                nc.tensor.matmul(
                    rbq[0:64, 0:w], ones1[:], st["rc"][:, 0:w],
                    start=True, stop=True,
                )
                st["rbq"] = rbq

            def e3():
                rb = rbp.tile([64, 512], bf16, tag="rb%d" % h, name="rb")
                # pin to ACT: recip (DVE) -> rbq (PE) -> rb (ACT) -> norm (DVE)
                # pipelines across engines instead of serializing on DVE
                bal.charge("act", w)
                nc.scalar.copy(rb[:, 0:w], st["rbq"][0:64, 0:w])
                st["rb"] = rb

            def e4():
                dst = y2[0:64, qcols] if h == 0 else y2B[0:64, qcols]
                nc.vector.tensor_tensor(
                    dst, yp[0:64, cw], st["rb"][:, 0:w], OP.mult
                )
                bal.charge("dve", w)
                if h == 1 and merge:
                    nc.sync.dma_start(y2[64:128, qcols], y2B[0:64, qcols])

            return [e1, e2, e3, e4]

        # ------------------------------------------------ attention section
        def section(b, qi, last=False):
            qcol0 = b * 2048 + qi * 512
            nj = 4 * qi + 4
            n_slots = 2 * (2 * nj + PIPE)

            def pump_ratio():
                # front-loaded: drain filler in the first ~60% of the section
                # so next-slab RoPE chains complete before the next section
                k = len(filler)
                slots = max(1, (st_slots[0] * 3) // 5)
                return max(1, -(-k // slots)) if k else 0

            st_slots = [n_slots]
            for h in range(2):
                yp = ypp.tile([65, 512], f32, tag="yp%d" % h, name="yp")
                # Diagonal (masked) tiles early so their exp+mask chain hides
                # mid-pipeline, but after two full tiles (old slabs) so the
                # section start doesn't wait on the newest slab's RoPE chain.
                fulls = list(range(0, nj - 4))
                diags = list(range(nj - 1, nj - 5, -1))
                cut = min(4, len(fulls))
                js = fulls[0:cut] + diags + fulls[cut:]
                inflight = []
                n_popped = [0]
                for idx in range(nj + PIPE):
                    if idx < nj:
                        j = js[idx]
                        dlt = j * 128 - qi * 512
                        dlt0 = max(dlt, 0)
                        w = 512 - dlt0
                        sp = spp.tile([128, 512], f32, tag="sp", name="sp")
                        nc.tensor.matmul(
                            sp[:, ds(dlt0, w)],
                            kT[64 * h : 64 * h + 64, ds(b * 2048 + j * 128, 128)],
                            qT[64 * h : 64 * h + 64, ds(qcol0 + dlt0, w)],
                            start=True, stop=True,
                        )
                        inflight.append((j, dlt, dlt0, sp))
                        st_slots[0] -= 1
                        pump(pump_ratio())
                    if len(inflight) >= PIPE or (idx >= nj and inflight):
                        j, dlt, dlt0, sp = inflight.pop(0)
                        w = 512 - dlt0
                        E = ep.tile([128, 512], bf16, tag="E", name="E")
                        nc.scalar.activation(
                            E[:, ds(dlt0, w)], sp[:, ds(dlt0, w)], AF.Exp
                        )
                        bal.charge("act", w)
                        if dlt >= 0:
                            nc.vector.tensor_tensor(
                                E[:, ds(dlt, 128)],
                                E[:, ds(dlt, 128)],
                                tri_t[:],
                                OP.mult,
                            )
                            bal.charge("dve", 128)
                        blk = (h * 2 + b) * 16 + j
                        nc.tensor.matmul(
                            yp[:, ds(dlt0, w)],
                            v_sb[:, blk, :],
                            E[:, ds(dlt0, w)],
                            start=(n_popped[0] == 0), stop=(n_popped[0] == nj - 1),
                            skip_group_check=True,
                        )
                        n_popped[0] += 1
                        st_slots[0] -= 1
                        pump(pump_ratio())
                if last:
                    if h == 0:
                        filler.extend(make_epilogue(b, qi, 0, yp, merge=False))
                    else:
                        ep_halves[0] = make_epilogue(b, qi, 1, yp, 0, 256, False)
                        ep_halves[1] = make_epilogue(b, qi, 1, yp, 256, 256, False)
                else:
                    filler.extend(make_epilogue(b, qi, h, yp))

        # ------------------------------------------------ schedule
        # startup: project slab (0,0) directly
        for c in make_proj(0, 0):
            c()

        ep_halves = {}
        chunks = [(b, qi) for b in range(2) for qi in range(4)]
        for sidx, (b, qi) in enumerate(chunks):
            # enqueue filler: next slab's projections + output projection of
            # the chunk two sections back (delayed so tail sections get work)
            if qi < 3:
                filler.extend(make_proj(b, qi + 1))
            elif b == 0:
                filler.extend(make_proj(1, 0))
            for oidx in [sidx - 2] + ([6] if sidx == 7 else []):
                if 0 <= oidx:
                    filler.extend([lambda: None] * 4)
                    filler.extend(make_oproj(*chunks[oidx]))
            section(b, qi, last=(b, qi) == (1, 3))
        while filler:
            pump()

        # final chunk: both piecewise h1 epilogue chains launched up-front so
        # they pipeline across DVE/PE/ACT, then split-contraction O-proj
        # pieces (head halves contracted separately; no y2 merge DMA needed),
        # copies alternating ACT/DVE, DMA fired per 128-token block
        qcol0 = 1 * 2048 + 3 * 512
        obL = obp.tile([128, 4, 1024], bf16, tag="ob", name="obL")
        chains = [list(ep_halves[0]), list(ep_halves[1])]
        for step in range(4):
            for half in range(2):
                chains[half][step]()
        for tcki in range(4):
            for oc in range(2):
                op = vopp.tile([128, 512], f32, tag="vop", name="opL")
                nc.tensor.matmul(
                    op[:],
                    y2[0:64, ds(qcol0 + tcki * 128, 128)],
                    woC[0:64, ds(oc * 512, 512)],
                    start=True, stop=False,
                )
                nc.tensor.matmul(
                    op[:],
                    y2B[0:64, ds(qcol0 + tcki * 128, 128)],
                    woC_lo[:, ds(oc * 512, 512)],
                    start=False, stop=True,
                )
                eng = "act" if oc == 0 else "dve"
                bal.charge(eng, 512)
                if eng == "act":
                    nc.scalar.copy(obL[:, tcki, ds(oc * 512, 512)], op[:])
                else:
                    nc.vector.tensor_copy(obL[:, tcki, ds(oc * 512, 512)], op[:])
            dst = (
                d["outp"][ds(qcol0 + tcki * 128, 128), :]
                .rearrange("(t p) o -> p t o", t=1)
            )
            nc.sync.dma_start(dst, obL[:, ds(tcki, 1), :])


_NC_CACHE = {}


def _build():
    if "nc" in _NC_CACHE:
        return _NC_CACHE["nc"]
    import concourse.bass as bass
    import concourse.mybir as mybir
    import concourse.tile as tile

    f32 = mybir.dt.float32
    bf16 = mybir.dt.bfloat16
    nc = bass.Bass("TRN2", target_bir_lowering=False, debug=False, num_devices=1)
    d = {
        "xt": nc.dram_tensor("xt", [8, 128, 8, 512], bf16, kind="ExternalInput").ap(),
        "wqT": nc.dram_tensor("wqT", [128, 8, 128], bf16, kind="ExternalInput").ap(),
        "wkT": nc.dram_tensor("wkT", [128, 8, 128], bf16, kind="ExternalInput").ap(),
        "wvT": nc.dram_tensor("wvT", [128, 8, 128], bf16, kind="ExternalInput").ap(),
        "woC": nc.dram_tensor("woC", [128, 1024], bf16, kind="ExternalInput").ap(),
        "cos_t": nc.dram_tensor("cos_t", [128, 2048], bf16, kind="ExternalInput").ap(),
        "sin_t": nc.dram_tensor("sin_t", [128, 2048], bf16, kind="ExternalInput").ap(),
        "tri": nc.dram_tensor("tri", [128, 128], bf16, kind="ExternalInput").ap(),
        "PT": nc.dram_tensor("PT", [128, 128], bf16, kind="ExternalInput").ap(),
        "ones1": nc.dram_tensor("ones1", [1, 64], bf16, kind="ExternalInput").ap(),
        "outp": nc.dram_tensor("outp", [4096, 1024], bf16, kind="ExternalOutput").ap(),
    }
    with tile.TileContext(nc) as tc:
        _emit(nc, tc, d)
    _split_excess_waits(nc)
    _NC_CACHE["nc"] = nc
    return nc


def kernel(x, wq, wk, wv, wo, rope_cos, rope_sin):
    from concourse import bass_utils

    x, wq, wk, wv, wo, rope_cos, rope_sin = (
        np.asarray(a, dtype=np.float32)
        for a in (x, wq, wk, wv, wo, rope_cos, rope_sin)
    )
    in_maps = _make_core_inputs(x, wq, wk, wv, wo, rope_cos, rope_sin)
    nc = _build()
    res = bass_utils.run_bass_kernel_spmd(nc, in_maps, core_ids=list(range(N_CORES)))
    total = np.zeros((B * T, C), np.float32)
    for i in range(N_CORES):
        total += res.results[i]["outp"].astype(np.float32)
    return total.reshape(B, T, C).astype(np.float32)


# revision 43
# speedup vs baseline: 1.0896x; 1.0430x over previous
"""Trainium2 Bass kernel for nn_CausalSelfAttention (BitLinear QKV/O + RoPE + causal attn).

Sharding: head-parallel, 2 heads x 2 batches per core; all matmul operands bf16
(fp32 PSUM accum). Single fused software pipeline: each attention chunk's
j-loop (scores -> exp -> E@V) is interleaved with "filler" work -- the next
slab's Q/K/V projections + RoPE and the previous chunk's output projection --
so the PE stream never drains. V is produced pre-transposed ([token, dim]) by
restructuring its projection (x-chunk as lhsT), eliminating PE transposes.
Diagonal (masked) tiles run first in reversed-j order so their exp+mask chain
hides under the pipeline fill. Q->K share one PSUM bank sequentially; scores
use a 3-deep PSUM rotation; exp runs on ACT; PSUM evacuation copies are
balanced ACT/DVE; RoPE cos/sin multiplies run on Pool. Partial outputs are
written bf16 and summed across cores on the host.
"""
import sys

sys.path.insert(0, "/opt/trn_rl_repo")

from collections import deque

import numpy as np

GROUP = 128
N_HEADS = 16
EPS = 1e-8
B, T, C = 2, 2048, 1024
HD = 64
N_CORES = 8
HPC = N_HEADS // N_CORES  # 2 heads per core
PIPE = 3


# ---------------------------------------------------------------- host prep
def _ternary_quantize(w):
    O, I = w.shape
    g = w.reshape(O, I // GROUP, GROUP).astype(np.float32)
    scale = np.maximum(np.mean(np.abs(g), axis=-1, keepdims=True), EPS).astype(
        np.float32
    )
    wn = g / scale
    q = np.where(wn > 0.5, 1.0, np.where(wn < -0.5, -1.0, 0.0)).astype(np.float32)
    return (q * scale).reshape(O, I).astype(np.float32)


def _np_bf16():
    import concourse.mybir as mybir

    return np.dtype(mybir.dt.np(mybir.dt.bfloat16))


def _np_fp8():
    import concourse.mybir as mybir

    return np.dtype(mybir.dt.np(mybir.dt.float8e4))


def _make_core_inputs(x, wq, wk, wv, wo, rope_cos, rope_sin):
    """Returns list of 8 per-core input dicts (bf16 device layouts)."""
    bf = _np_bf16()
    f8 = _np_fp8()
    x = np.ascontiguousarray(x.astype(np.float32).reshape(B * T, C))
    # Q/K projections run in fp8e4m3 DoubleRow: weights scaled x64 so group
    # scales sit in fp8's normal range; the 1/(64*64) and the attention scale
    # HD**-0.5 are folded into the exp's scale parameter on device.
    wq_q = _ternary_quantize(wq) * np.float32(64.0)
    wk_q = _ternary_quantize(wk) * np.float32(64.0)
    wv_q = _ternary_quantize(wv)
    wo_q = _ternary_quantize(wo)

    xT = x.T  # [1024 c, 4096 t]
    xt_f = np.ascontiguousarray(
        xT.reshape(8, 128, 8, 512).transpose(2, 1, 0, 3)
    )  # [s, p, cc, u]
    xt_slab = xt_f.astype(bf)
    xt8_slab = xt_f.astype(f8)

    cosT = rope_cos.astype(np.float32).T  # [32, 2048]
    sinT = rope_sin.astype(np.float32).T
    cos_t = np.tile(cosT, (4, 1)).astype(bf)
    sin_t = np.concatenate([-sinT, sinT, -sinT, sinT], axis=0).astype(bf)
    tri = (np.arange(128)[None, :] >= np.arange(128)[:, None]).astype(bf)
    # partition-swap matrix: out = PT^T @ in, out[m] = in[sigma(m)],
    # sigma swaps 32-row halves within each 64-row head block.
    m = np.arange(128)
    sigma = np.where(m % 64 < 32, m + 32, m - 32)
    PT = np.zeros((128, 128), np.float32)
    PT[sigma, m] = 1.0
    PT = PT.astype(bf)

    maps = []
    for core in range(N_CORES):
        r0 = core * HPC * HD
        rows = slice(r0, r0 + HPC * HD)

        def w_lhsT(w_qq, dt):
            wsT = w_qq[rows, :].T  # [1024 in, 128 d]
            return np.ascontiguousarray(
                wsT.reshape(8, 128, 128).transpose(1, 0, 2)
            ).astype(dt)

        maps.append(
            {
                "xt": xt_slab,
                "xt8": xt8_slab,
                "wqT": w_lhsT(wq_q, f8),
                "wkT": w_lhsT(wk_q, f8),
                "wvT": w_lhsT(wv_q, bf),
                "woC": np.ascontiguousarray(wo_q[:, rows].T).astype(bf),
                "cos_t": cos_t,
                "sin_t": sin_t,
                "tri": tri,
                "PT": PT,
                "ones1": np.ones((1, 64), np.float32).astype(bf),
            }
        )
    return maps


# ---------------------------------------------------------------- BIR post-pass
def _split_excess_waits(nc, max_waits=1):
    """walrus CoreV3 codegen rejects instructions with >1 sem wait; split the
    excess into preceding NoOps on the same engine."""
    import concourse.mybir as mybir

    for f in nc.m.functions:
        for bb in f.blocks:
            insts = bb.instructions
            i = 0
            while i < len(insts):
                ins = insts[i]
                si = ins.sync_info
                if si is not None and si.on_wait and len(si.on_wait) > max_waits:
                    waits = list(si.on_wait)
                    si.on_wait = waits[:max_waits]
                    rest = waits[max_waits:]
                    new_ops = []
                    for j in range(0, len(rest), max_waits):
                        new_ops.append(
                            mybir.InstNoOp(
                                name=nc.get_next_instruction_name(),
                                sync_info=mybir.SyncInfo(
                                    on_wait=rest[j : j + max_waits], on_update=[]
                                ),
                                bass_nofuse=True,
                                engine=ins.engine,
                            )
                        )
                    insts[i:i] = new_ops
                    i += len(new_ops)
                i += 1


# ---------------------------------------------------------------- device kernel
class _Balancer:
    """Greedy least-loaded assignment of PSUM-evacuation copies to ACT/DVE."""

    def __init__(self):
        self.busy = {"act": 0.0, "dve": 0.0}

    @staticmethod
    def cost(eng, free):
        if eng == "act":
            return free * 0.833 + 185.0
        return free * 1.042 + 125.0

    def charge(self, eng, free):
        self.busy[eng] += self.cost(eng, free)
        return eng

    def pick(self, free, bias=None):
        bias = bias or {}
        best = min(
            ("act", "dve"),
            key=lambda e: self.busy[e] + self.cost(e, free) + bias.get(e, 0.0),
        )
        return self.charge(best, free)


def _emit(nc, tc, d):
    import concourse.mybir as mybir
    from concourse.bass import ds

    f32 = mybir.dt.float32
    bf16 = mybir.dt.bfloat16
    AF = mybir.ActivationFunctionType
    OP = mybir.AluOpType
    bal = _Balancer()

    def emit_copy(dst, src, free, bias=None):
        eng = bal.pick(free, bias)
        if eng == "act":
            nc.scalar.copy(dst, src)
        else:
            nc.vector.tensor_copy(dst, src)

    with nc.allow_low_precision(
        reason="bf16 matmul operands; fp32 accum in PSUM"
    ), tc.tile_pool(name="const", bufs=1) as cp, tc.tile_pool(
        name="persist", bufs=1
    ) as pp, tc.tile_pool(name="xt", bufs=4) as xtp, tc.tile_pool(
        name="raw", bufs=2
    ) as rawp, tc.tile_pool(name="rope", bufs=2) as ropep, tc.tile_pool(
        name="E", bufs=6
    ) as ep, tc.tile_pool(name="rc", bufs=2) as rcp, tc.tile_pool(
        name="rb", bufs=2
    ) as rbp, tc.tile_pool(name="ob", bufs=2) as obp, tc.tile_pool(
        name="vop", bufs=3, space="PSUM"
    ) as vopp, tc.tile_pool(
        name="yp", bufs=1, space="PSUM"
    ) as ypp, tc.tile_pool(name="sp", bufs=3, space="PSUM") as spp:
        # ---- constants / inputs: K weights + x first (K projects first),
        # spread across the SP/ACT/DVE DMA rings so issue overhead overlaps
        fp8 = mybir.dt.float8e4
        wk_t = cp.tile([128, 8, 128], fp8)
        nc.sync.dma_start(wk_t[:, 0:2, :], d["wkT"][:, 0:2, :])
        xt80 = xtp.tile([128, 8, 512], fp8, tag="xt8", name="xt80")
        nc.sync.dma_start(xt80[:, 0:4, :], d["xt8"][0][:, 0:4, :])
        nc.sync.dma_start(wk_t[:, 2:8, :], d["wkT"][:, 2:8, :])
        nc.sync.dma_start(xt80[:, 4:8, :], d["xt8"][0][:, 4:8, :])
        wq_t = cp.tile([128, 8, 128], fp8)
        nc.scalar.dma_start(wq_t[:], d["wqT"])
        wv_t = cp.tile([128, 8, 128], bf16)
        nc.scalar.dma_start(wv_t[:], d["wvT"])
        xt0 = xtp.tile([128, 8, 512], bf16, tag="xt", name="xt0")
        nc.sync.dma_start(xt0[:], d["xt"][0])
        xt81 = xtp.tile([128, 8, 512], fp8, tag="xt8", name="xt81")
        nc.sync.dma_start(xt81[:], d["xt8"][1])
        xt1 = xtp.tile([128, 8, 512], bf16, tag="xt", name="xt1")
        nc.sync.dma_start(xt1[:, 0:4, :], d["xt"][1][:, 0:4, :])
        nc.sync.dma_start(xt1[:, 4:8, :], d["xt"][1][:, 4:8, :])
        PT_t = cp.tile([128, 128], bf16)
        nc.scalar.dma_start(PT_t[:], d["PT"])
        cos_sb = cp.tile([128, 2048], bf16)
        nc.scalar.dma_start(cos_sb[:], d["cos_t"])
        sin_sb = cp.tile([128, 2048], bf16)
        nc.scalar.dma_start(sin_sb[:], d["sin_t"])
        tri_t = cp.tile([128, 128], bf16)
        nc.scalar.dma_start(tri_t[:], d["tri"])
        woC = cp.tile([128, 1024], bf16)
        nc.sync.dma_start(woC[:], d["woC"])
        woC_lo = cp.tile([64, 1024], bf16)
        nc.sync.dma_start(woC_lo[:], d["woC"][64:128, :])
        ones1 = cp.tile([1, 64], bf16)
        nc.sync.dma_start(ones1[:], d["ones1"])

        qT = pp.tile([128, 4096], bf16)
        kT = pp.tile([128, 4096], bf16)
        v_sb = pp.tile([128, 64, 65], bf16)
        nc.gpsimd.memset(v_sb[:, :, 64:65], 1.0)  # denominator ones column
        y2 = pp.tile([128, 4096], bf16)
        y2B = pp.tile([64, 4096], bf16)

        xt_tiles = {0: xt0, 1: xt1}
        xt8_tiles = {0: xt80, 1: xt81}

        def load_slab(s):
            t8 = xtp.tile([128, 8, 512], fp8, tag="xt8", name="xt8%d" % s)
            nc.sync.dma_start(t8[:], d["xt8"][s])
            xt8_tiles[s] = t8
            t = xtp.tile([128, 8, 512], bf16, tag="xt", name="xt%d" % s)
            nc.sync.dma_start(t[:], d["xt"][s])
            xt_tiles[s] = t

        filler = deque()

        def pump(k=1):
            for _ in range(k):
                if filler:
                    filler.popleft()()

        # ------------------------------------------------ projection closures
        def make_proj(b, ci):
            """Closure list: K then Q proj (shared PSUM tag, sequential), V^T
            proj, RoPE. Emitted as filler inside the previous attention
            section. K first so its longer RoPE chain hides under Q/V mms."""
            s = b * 4 + ci
            cls = []
            if s + 2 < 8:
                cls.append(lambda s2=s + 2: load_slab(s2))

            dcol = ds(s * 512, 512)
            lcol = ds(ci * 512, 512)

            def qk_closures(nm, w_t, dest):
                ps = vopp.tile([128, 512], f32, tag="vop", name="prj" + nm)

                def mk_mm(pair, dh):
                    def go():
                        nc.tensor.matmul(
                            ps[ds(64 * dh, 64), :],
                            w_t[:, ds(2 * pair, 2), ds(64 * dh, 64)],
                            xt8_tiles[s][:, ds(2 * pair, 2), :],
                            start=(pair == 0), stop=(pair == 3),
                            perf_mode=mybir.MatmulPerfMode.DoubleRow,
                        )

                    return go

                mms = [mk_mm(p, dh) for dh in range(2) for p in range(4)]
                raw = rawp.tile([128, 512], bf16, tag="raw" + nm, name="raw" + nm)

                def copy_raw():
                    emit_copy(raw[:], ps[:], 512)

                swp = vopp.tile([128, 512], f32, tag="vop", name="swp" + nm)

                def do_swap():
                    nc.tensor.matmul(swp[:], PT_t[:], raw[:], start=True, stop=True)

                t1 = ropep.tile([128, 512], bf16, tag="t1" + nm)
                t2 = ropep.tile([128, 512], bf16, tag="t2" + nm)

                def r1():
                    nc.gpsimd.tensor_tensor(t1[:], raw[:], cos_sb[:, lcol], OP.mult)

                def r2():
                    nc.vector.tensor_tensor(t2[:], swp[:], sin_sb[:, lcol], OP.mult)
                    bal.charge("dve", 512)

                def r3():
                    nc.vector.tensor_tensor(dest[:, dcol], t1[:], t2[:], OP.add)
                    bal.charge("dve", 256)  # bf16 2x mode

                return mms, copy_raw, do_swap, [r1, r2, r3]

            kmm, kcopy, kswap, krope = qk_closures("k", wk_t, kT)
            qmm, qcopy, qswap, qrope = qk_closures("q", wq_t, qT)

            # V^T: out [128 t, 128 d] per 128-token block, lhsT = x chunk
            pv = vopp.tile([128, 4, 128], f32, tag="vop", name="pv")

            vcls = []
            for tblk in range(4):
                for jh in range(2):

                    def go(tblk=tblk, jh=jh):
                        for j in range(jh * 4, jh * 4 + 4):
                            nc.tensor.matmul(
                                pv[:, tblk, :],
                                xt_tiles[s][:, j, ds(tblk * 128, 128)],
                                wv_t[:, j, :],
                                start=(j == 0), stop=(j == 7),
                            )

                    vcls.append(go)

            def mk_vcopy(h):
                def go():
                    blk0 = (h * 2 + b) * 16 + ci * 4
                    emit_copy(
                        v_sb[:, ds(blk0, 4), 0:64],
                        pv[:, :, ds(h * 64, 64)],
                        256,
                    )

                return go

            cls.extend(kmm)
            cls.append(kcopy)
            cls.extend(vcls[0:2])
            cls.append(kswap)
            cls.append(krope[0])
            cls.append(krope[1])
            cls.append(krope[2])
            cls.extend(qmm)
            cls.append(qcopy)
            cls.extend(vcls[2:4])
            cls.append(qswap)
            cls.append(qrope[0])
            cls.append(qrope[1])
            cls.append(qrope[2])
            cls.extend(vcls[4:8])
            cls.append(mk_vcopy(0))
            cls.append(mk_vcopy(1))
            return cls

        # ------------------------------------------------ output projection
        def make_oproj(b, qi):
            qcol0 = b * 2048 + qi * 512
            ob = obp.tile([128, 4, 1024], bf16, tag="ob", name="ob")
            state = {"n": 0}

            def piece(tcki, oc):
                def go():
                    op = vopp.tile([128, 512], f32, tag="vop", name="op")
                    nc.tensor.matmul(
                        op[:],
                        y2[:, ds(qcol0 + tcki * 128, 128)],
                        woC[:, ds(oc * 512, 512)],
                        start=True, stop=True,
                    )
                    emit_copy(ob[:, tcki, ds(oc * 512, 512)], op[:], 512)
                    state["n"] += 1
                    if state["n"] == 8:
                        dst = (
                            d["outp"][ds(qcol0, 512), :]
                            .rearrange("(t p) o -> p t o", t=4)
                        )
                        nc.sync.dma_start(dst, ob[:])

                return go

            return [piece(t, o) for t in range(4) for o in range(2)]

        # ------------------------------------------------ epilogue (normalize)
        def make_epilogue(b, qi, h, yp, c0=0, w=512, merge=True):
            qcols = ds(b * 2048 + qi * 512 + c0, w)
            cw = ds(c0, w)
            st = {}

            def e1():
                rc = rcp.tile([1, 512], bf16, tag="rc%d" % h, name="rc")
                nc.vector.reciprocal(rc[:, 0:w], yp[64:65, cw])
                bal.charge("dve", w)
                st["rc"] = rc

            def e2():
                rbq = spp.tile([128, 512], f32, tag="sp", name="rbq")
                nc.tensor.matmul(
                    rbq[0:64, 0:w], ones1[:], st["rc"][:, 0:w],
                    start=True, stop=True,
                )
                st["rbq"] = rbq

            def e3():
                rb = rbp.tile([64, 512], bf16, tag="rb%d" % h, name="rb")
                # pin to ACT: recip (DVE) -> rbq (PE) -> rb (ACT) -> norm (DVE)
                # pipelines across engines instead of serializing on DVE
                bal.charge("act", w)
                nc.scalar.copy(rb[:, 0:w], st["rbq"][0:64, 0:w])
                st["rb"] = rb

            def e4():
                dst = y2[0:64, qcols] if h == 0 else y2B[0:64, qcols]
                nc.vector.tensor_tensor(
                    dst, yp[0:64, cw], st["rb"][:, 0:w], OP.mult
                )
                bal.charge("dve", w)
                if h == 1 and merge:
                    nc.sync.dma_start(y2[64:128, qcols], y2B[0:64, qcols])

            return [e1, e2, e3, e4]

        # ------------------------------------------------ attention section
        def section(b, qi, last=False):
            qcol0 = b * 2048 + qi * 512
            nj = 4 * qi + 4
            n_slots = 2 * (2 * nj + PIPE)

            def pump_ratio():
                # front-loaded: drain filler in the first ~60% of the section
                # so next-slab RoPE chains complete before the next section
                k = len(filler)
                slots = max(1, (st_slots[0] * 3) // 5)
                return max(1, -(-k // slots)) if k else 0

            st_slots = [n_slots]
            for h in range(2):
                yp = ypp.tile([65, 512], f32, tag="yp%d" % h, name="yp")
                # Diagonal (masked) tiles early so their exp+mask chain hides
                # mid-pipeline, but after two full tiles (old slabs) so the
                # section start doesn't wait on the newest slab's RoPE chain.
                fulls = list(range(0, nj - 4))
                diags = list(range(nj - 1, nj - 5, -1))
                cut = min(4, len(fulls))
                js = fulls[0:cut] + diags + fulls[cut:]
                inflight = []
                n_popped = [0]
                for idx in range(nj + PIPE):
                    if idx < nj:
                        j = js[idx]
                        dlt = j * 128 - qi * 512
                        dlt0 = max(dlt, 0)
                        w = 512 - dlt0
                        sp = spp.tile([128, 512], f32, tag="sp", name="sp")
                        nc.tensor.matmul(
                            sp[:, ds(dlt0, w)],
                            kT[64 * h : 64 * h + 64, ds(b * 2048 + j * 128, 128)],
                            qT[64 * h : 64 * h + 64, ds(qcol0 + dlt0, w)],
                            start=True, stop=True,
                        )
                        inflight.append((j, dlt, dlt0, sp))
                        st_slots[0] -= 1
                        pump(pump_ratio())
                    if len(inflight) >= PIPE or (idx >= nj and inflight):
                        j, dlt, dlt0, sp = inflight.pop(0)
                        w = 512 - dlt0
                        E = ep.tile([128, 512], bf16, tag="E", name="E")
                        nc.scalar.activation(
                            E[:, ds(dlt0, w)], sp[:, ds(dlt0, w)], AF.Exp,
                            scale=float(HD**-0.5 / 4096.0),
                        )
                        bal.charge("act", w)
                        if dlt >= 0:
                            nc.vector.tensor_tensor(
                                E[:, ds(dlt, 128)],
                                E[:, ds(dlt, 128)],
                                tri_t[:],
                                OP.mult,
                            )
                            bal.charge("dve", 128)
                        blk = (h * 2 + b) * 16 + j
                        nc.tensor.matmul(
                            yp[:, ds(dlt0, w)],
                            v_sb[:, blk, :],
                            E[:, ds(dlt0, w)],
                            start=(n_popped[0] == 0), stop=(n_popped[0] == nj - 1),
                            skip_group_check=True,
                        )
                        n_popped[0] += 1
                        st_slots[0] -= 1
                        pump(pump_ratio())
                if last:
                    if h == 0:
                        filler.extend(make_epilogue(b, qi, 0, yp, merge=False))
                    else:
                        ep_halves[0] = make_epilogue(b, qi, 1, yp, 0, 256, False)
                        ep_halves[1] = make_epilogue(b, qi, 1, yp, 256, 256, False)
                else:
                    filler.extend(make_epilogue(b, qi, h, yp))

        # ------------------------------------------------ schedule
        # startup: project slab (0,0) directly
        for c in make_proj(0, 0):
            c()

        ep_halves = {}
        chunks = [(b, qi) for b in range(2) for qi in range(4)]
        for sidx, (b, qi) in enumerate(chunks):
            # enqueue filler: next slab's projections + output projection of
            # the chunk two sections back (delayed so tail sections get work)
            if sidx + 1 < 8:
                filler.extend(make_proj(*chunks[sidx + 1]))
            for oidx in [sidx - 2] + ([6] if sidx == 7 else []):
                if 0 <= oidx:
                    filler.extend([lambda: None] * 4)
                    filler.extend(make_oproj(*chunks[oidx]))
            section(b, qi, last=(b, qi) == (1, 3))
        while filler:
            pump()

        # final chunk: both piecewise h1 epilogue chains launched up-front so
        # they pipeline across DVE/PE/ACT, then split-contraction O-proj
        # pieces (head halves contracted separately; no y2 merge DMA needed),
        # copies alternating ACT/DVE, DMA fired per 128-token block
        qcol0 = 1 * 2048 + 3 * 512
        obL = obp.tile([128, 4, 1024], bf16, tag="ob", name="obL")
        chains = [list(ep_halves[0]), list(ep_halves[1])]
        for step in range(4):
            for half in range(2):
                chains[half][step]()
        for tcki in range(4):
            for oc in range(2):
                op = vopp.tile([128, 512], f32, tag="vop", name="opL")
                nc.tensor.matmul(
                    op[:],
                    y2[0:64, ds(qcol0 + tcki * 128, 128)],
                    woC[0:64, ds(oc * 512, 512)],
                    start=True, stop=False,
                )
                nc.tensor.matmul(
                    op[:],
                    y2B[0:64, ds(qcol0 + tcki * 128, 128)],
                    woC_lo[:, ds(oc * 512, 512)],
                    start=False, stop=True,
                )
                eng = "act" if oc == 0 else "dve"
                bal.charge(eng, 512)
                if eng == "act":
                    nc.scalar.copy(obL[:, tcki, ds(oc * 512, 512)], op[:])
                else:
                    nc.vector.tensor_copy(obL[:, tcki, ds(oc * 512, 512)], op[:])
            dst = (
                d["outp"][ds(qcol0 + tcki * 128, 128), :]
                .rearrange("(t p) o -> p t o", t=1)
            )
            nc.sync.dma_start(dst, obL[:, ds(tcki, 1), :])


_NC_CACHE = {}


def _build():
    if "nc" in _NC_CACHE:
        return _NC_CACHE["nc"]
    import concourse.bass as bass
    import concourse.mybir as mybir
    import concourse.tile as tile

    f32 = mybir.dt.float32
    bf16 = mybir.dt.bfloat16
    nc = bass.Bass("TRN2", target_bir_lowering=False, debug=False, num_devices=1)
    d = {
        "xt": nc.dram_tensor("xt", [8, 128, 8, 512], bf16, kind="ExternalInput").ap(),
        "xt8": nc.dram_tensor(
            "xt8", [8, 128, 8, 512], mybir.dt.float8e4, kind="ExternalInput"
        ).ap(),
        "wqT": nc.dram_tensor(
            "wqT", [128, 8, 128], mybir.dt.float8e4, kind="ExternalInput"
        ).ap(),
        "wkT": nc.dram_tensor(
            "wkT", [128, 8, 128], mybir.dt.float8e4, kind="ExternalInput"
        ).ap(),
        "wvT": nc.dram_tensor("wvT", [128, 8, 128], bf16, kind="ExternalInput").ap(),
        "woC": nc.dram_tensor("woC", [128, 1024], bf16, kind="ExternalInput").ap(),
        "cos_t": nc.dram_tensor("cos_t", [128, 2048], bf16, kind="ExternalInput").ap(),
        "sin_t": nc.dram_tensor("sin_t", [128, 2048], bf16, kind="ExternalInput").ap(),
        "tri": nc.dram_tensor("tri", [128, 128], bf16, kind="ExternalInput").ap(),
        "PT": nc.dram_tensor("PT", [128, 128], bf16, kind="ExternalInput").ap(),
        "ones1": nc.dram_tensor("ones1", [1, 64], bf16, kind="ExternalInput").ap(),
        "outp": nc.dram_tensor("outp", [4096, 1024], bf16, kind="ExternalOutput").ap(),
    }
    with tile.TileContext(nc) as tc:
        _emit(nc, tc, d)
    _split_excess_waits(nc)
    _NC_CACHE["nc"] = nc
    return nc


def kernel(x, wq, wk, wv, wo, rope_cos, rope_sin):
    from concourse import bass_utils

    x, wq, wk, wv, wo, rope_cos, rope_sin = (
        np.asarray(a, dtype=np.float32)
        for a in (x, wq, wk, wv, wo, rope_cos, rope_sin)
    )
    in_maps = _make_core_inputs(x, wq, wk, wv, wo, rope_cos, rope_sin)
    nc = _build()
    res = bass_utils.run_bass_kernel_spmd(nc, in_maps, core_ids=list(range(N_CORES)))
    total = np.zeros((B * T, C), np.float32)
    for i in range(N_CORES):
        total += res.results[i]["outp"].astype(np.float32)
    return total.reshape(B, T, C).astype(np.float32)
